# revision 1
# baseline (speedup 1.0000x reference)
"""Distributed Trainium2 kernel for quantized-mixed int8 matmul dequant.

Reference computation (M = K = N = 4096):
    xf = (x - X_ZP) * X_SCALE      # x int32 values in [-128, 127]
    yf = (y - Y_ZP) * Y_SCALE      # y int32 values in [0, 255]
    out = xf @ yf                  # float32 [M, N]

Strategy: 2D-shard the GEMM over 8 NeuronCores as a 4x2 grid
(M split 4 ways, N split 2 ways -> per-core C tile of 1024 x 2048).
Per core, the dequant is fused on-chip: int32 shards are DMA'd in,
shifted by the zero point and cast to bf16 (exact: all shifted values
are integers < 256, exactly representable in bf16), then accumulated
over K in fp32 PSUM via the TensorEngine; the combined scale
X_SCALE*Y_SCALE is applied in the PSUM->SBUF epilogue copy.

x is fed pre-transposed ([K, Mc] int32) so its k-chunks are the
matmul's stationary operand without any on-device transpose.
"""

import numpy as np

import concourse.bacc as bacc
import concourse.mybir as mybir
import concourse.tile as tile
from concourse.bass_utils import run_bass_kernel_spmd

M = K = N = 4096
X_SCALE, X_ZP = 0.03, -66
Y_SCALE, Y_ZP = 0.025, 160
OUT_SCALE = X_SCALE * Y_SCALE

NCORES = 8
MSPLIT, NSPLIT = 4, 2
MC = M // MSPLIT          # 1024 rows of C per core
NCOLS = N // NSPLIT       # 2048 cols of C per core
P = 128                   # partitions / k-chunk size
KC = K // P               # 32 k-chunks
MT = MC // P              # 8 m-tiles (one PSUM bank each)
NF = 512                  # matmul free dim (one PSUM bank at fp32)
NG = NCOLS // NF          # 4 n-groups

_CACHE = {}


def _build():
    nc = bacc.Bacc("TRN2", target_bir_lowering=False, debug=False)
    xt = nc.dram_tensor("xt", [K, MC], mybir.dt.int32, kind="ExternalInput")
    # y and out are fed/returned in n-group-blocked layout so every chunk
    # DMA is fully contiguous (256KB) instead of 128 strided 2KB rows.
    y = nc.dram_tensor("y", [NG, K, NF], mybir.dt.int32, kind="ExternalInput")
    out = nc.dram_tensor("out", [NG, MC, NF], mybir.dt.float32, kind="ExternalOutput")

    with tile.TileContext(nc) as tc:
        with (
            tc.tile_pool(name="warm_pool", bufs=1) as warm_pool,
            tc.tile_pool(name="xs_pool", bufs=3) as xs_pool,
            tc.tile_pool(name="xb_pool", bufs=KC) as xb_pool,
            tc.tile_pool(name="ys_pool", bufs=14) as ys_pool,
            tc.tile_pool(name="yb_pool", bufs=36) as yb_pool,
            tc.tile_pool(name="ot_pool", bufs=16) as ot_pool,
            tc.tile_pool(name="ps_pool", bufs=8, space="PSUM") as ps_pool,
        ):
            # PE warm-up: the first ~10us of the kernel are DMA/convert
            # latency with no matmul work, which leaves the PE clock
            # throttled (HAM cold, 1.2 GHz). Burn dummy matmuls on a
            # zeroed tile during that window so the HAM un-throttles
            # and the first real matmuls issue at 2.4 GHz.
            wt = warm_pool.tile([P, NF], mybir.dt.bfloat16, tag="wt")
            nc.vector.memset(wt[:], 0.0)
            wps = ps_pool.tile([64, NF], mybir.dt.float32, tag="ps", name="wps")
            # 12 x ~0.43-0.51us keeps the PE busy past one full HAM SHORT
            # window (~3.4us) without delaying the first real matmuls.
            for _ in range(12):
                nc.tensor.matmul(wps[:], wt[:, :64], wt[:], start=True, stop=True)
            def load_y_chunk(g, k):
                ys = ys_pool.tile([P, NF], mybir.dt.int32, tag="ys",
                                  name=f"ys{g}_{k}")
                nc.sync.dma_start(out=ys[:], in_=y[g, k * P:(k + 1) * P, :])
                yb = yb_pool.tile([P, NF], mybir.dt.bfloat16, tag="yb",
                                  name=f"yb{g}_{k}")
                nc.vector.tensor_scalar_add(out=yb[:], in0=ys[:],
                                            scalar1=float(-Y_ZP))
                return yb

            PF = 4  # next-group chunks hoisted ahead of the epilogues
            xbf = [None] * KC
            prefetched = {}
            for g in range(NG - 1):
                psums = [None] * MT
                for k in range(KC):
                    if g == 0:
                        # Stream x in once; converted bf16 chunks stay
                        # resident in SBUF for all n-groups.
                        xs = xs_pool.tile([P, MC], mybir.dt.int32, tag="xs",
                                          name=f"xs{k}")
                        nc.sync.dma_start(out=xs[:], in_=xt[k * P:(k + 1) * P, :])
                        xb = xb_pool.tile([P, MC], mybir.dt.bfloat16, tag="xb",
                                          name=f"xb{k}")
                        nc.vector.tensor_scalar_add(out=xb[:], in0=xs[:],
                                                    scalar1=float(-X_ZP))
                        xbf[k] = xb
                    yb = prefetched.pop((g, k), None)
                    if yb is None:
                        yb = load_y_chunk(g, k)
                    for m in range(MT):
                        if k == 0:
                            psums[m] = ps_pool.tile([P, NF], mybir.dt.float32,
                                                    tag="ps", name=f"ps{g}_{m}")
                        nc.tensor.matmul(psums[m][:],
                                         xbf[k][:, m * P:(m + 1) * P],
                                         yb[:],
                                         start=(k == 0), stop=(k == KC - 1))
                # Hoist the next group's first chunks ahead of the epilogue
                # copies so the DVE isn't head-of-line blocked converting
                # them behind 4 PSUM-drain copies at the group boundary.
                npf = KC if g + 2 == NG else PF  # last group: hoist ALL chunks
                for k in range(npf):
                    prefetched[(g + 1, k)] = load_y_chunk(g + 1, k)
                for m in range(MT):
                    ot = ot_pool.tile([P, NF], mybir.dt.float32, tag="ot",
                                      name=f"ot{g}_{m}")
                    # Scale fused into the PSUM->SBUF copy; alternate
                    # engines so bank release isn't serialized on one.
                    if m % 2 == 0:
                        nc.scalar.mul(ot[:], psums[m][:], OUT_SCALE)
                    else:
                        nc.vector.tensor_scalar_mul(out=ot[:], in0=psums[m][:],
                                                    scalar1=OUT_SCALE)
                    # Output DMA on the gpsimd queue: on the sync queue its
                    # embedded wait (for the epilogue copy) head-of-line
                    # blocks the next group's y DMA triggers. Exception:
                    # the final group's outs go on the fast sync/vector
                    # HWDGE queues (idle by then, split so the trigger
                    # serialization doesn't stack) — the gpsimd SWDGE's
                    # ~6us latency would sit on the kernel tail.
                    nc.gpsimd.dma_start(
                        out=out[g, m * P:(m + 1) * P, :],
                        in_=ot[:])

            # Final group: m-outer / k-inner over the fully-prefetched y
            # half, so each m-tile's epilogue + output DMA stagger across
            # the group instead of bunching into the kernel tail. Outs go
            # on the fast sync/scalar HWDGE queues (idle by now).
            g = NG - 1
            ybs = [prefetched.pop((g, k)) for k in range(KC)]
            for m in range(MT):
                psum = ps_pool.tile([P, NF], mybir.dt.float32, tag="ps",
                                    name=f"psL_{m}")
                for k in range(KC):
                    nc.tensor.matmul(psum[:],
                                     xbf[k][:, m * P:(m + 1) * P],
                                     ybs[k][:],
                                     start=(k == 0), stop=(k == KC - 1))
                ot = ot_pool.tile([P, NF], mybir.dt.float32, tag="ot",
                                  name=f"otL_{m}")
                if m % 2 == 0:
                    nc.scalar.mul(ot[:], psum[:], OUT_SCALE)
                else:
                    nc.vector.tensor_scalar_mul(out=ot[:], in0=psum[:],
                                                scalar1=OUT_SCALE)
                dma_eng = nc.sync if m % 2 == 0 else nc.scalar
                dma_eng.dma_start(out=out[g, m * P:(m + 1) * P, :], in_=ot[:])
    nc.compile()
    return nc


def _get_nc():
    if "nc" not in _CACHE:
        _CACHE["nc"] = _build()
    return _CACHE["nc"]


def _shard(x, y):
    x = np.ascontiguousarray(np.asarray(x, dtype=np.int32))
    y = np.ascontiguousarray(np.asarray(y, dtype=np.int32))
    xts = [np.ascontiguousarray(x[mi * MC:(mi + 1) * MC, :].T)
           for mi in range(MSPLIT)]
    # n-group-blocked y: [NG, K, NF], so device chunk DMAs are contiguous
    ys = [np.ascontiguousarray(
              y[:, ni * NCOLS:(ni + 1) * NCOLS].reshape(K, NG, NF)
              .transpose(1, 0, 2))
          for ni in range(NSPLIT)]
    in_maps = []
    for c in range(NCORES):
        mi, ni = divmod(c, NSPLIT)
        in_maps.append({"xt": xts[mi], "y": ys[ni]})
    return in_maps


def _gather(results):
    out = np.empty((M, N), dtype=np.float32)
    for c in range(NCORES):
        mi, ni = divmod(c, NSPLIT)
        blk = results[c]["out"]  # [NG, MC, NF] group-blocked
        out[mi * MC:(mi + 1) * MC, ni * NCOLS:(ni + 1) * NCOLS] = \
            blk.transpose(1, 0, 2).reshape(MC, NCOLS)
    return out


def run(x, y, **spmd_kwargs):
    """Run and return (full_output, BassKernelResults)."""
    nc = _get_nc()
    in_maps = _shard(x, y)
    res = run_bass_kernel_spmd(nc, in_maps, core_ids=list(range(NCORES)),
                               **spmd_kwargs)
    return _gather(res.results), res


def kernel(x, y):
    out, _ = run(x, y)
    return out



# revision 3
# speedup vs baseline: 1.0123x; 1.0123x over previous
"""Distributed Trainium2 kernel for quantized-mixed int8 matmul dequant.

Reference computation (M = K = N = 4096):
    xf = (x - X_ZP) * X_SCALE      # x int32 values in [-128, 127]
    yf = (y - Y_ZP) * Y_SCALE      # y int32 values in [0, 255]
    out = xf @ yf                  # float32 [M, N]

Strategy: 2D-shard the GEMM over 8 NeuronCores as a 2x4 grid
(M split 2 ways, N split 4 ways -> per-core C tile of 2048 x 1024),
with the matmul run in fp8 (E4M3) DoubleRow mode (double-pumped PE:
two k-rows per cell per pass, 2x bf16 matmul throughput; measured
216ns per [256k x 128m x 512n] matmul = the fp8 roofline, 110.6us
of matmul work per core).

fp8 precision scheme (rel err ~7e-3 vs the 2e-2 gate):
  x is centered:  x~ = (x + 0.5) * sqrt(S)   in [-127.5, 127.5]*sqrt(S)
  y is shifted:   y~ = (y - 160) * sqrt(S)   in [-160, 95]*sqrt(S)
  out[m,n] = sum_k x~ y~ + g[n],  g[n] = 65.5 * S * colsum(y - 160)[n]
The sqrt(S) prescale (S = X_SCALE*Y_SCALE) keeps fp8 relative precision
identical while making PSUM hold final-scale values, so the epilogue is
a single tensor_add of the exact (host-computed, fp32) g correction.
Centering x halves its top-end quantization step (128..193 would round
at step 16; +-127.5 rounds at step 8). fp8 e4m3 products are exact in
the PE's e10m10 intermediate, so a host numpy simulation of the fp8
rounding predicts the HW result bit-for-bit (verified: 6.991e-03 both).

Both fp8 operand shards are SBUF-resident (x 8MB + y 4MB of ~26MB), so
after the upload ramp the PE runs with zero DMA waits. Upload layout is
chosen so the ramp critical path is short: the first m-tile sweep needs
ALL of y but only the first m-half of x, so y is the SMALL shard (4MB,
~20us on one ring) and x's two m-halves upload in order on the other
ring. Each ring carries one sequential HBM stream — interleaving x/y
chunks across both rings makes 4 concurrent HBM streams, halves upload
bandwidth, stalls the PE >5us, and the HAM clock gate then runs the PE
~1.2x slow for the WHOLE kernel (216 -> 259ns per matmul, measured).
Per-double-chunk DMAs (256KB) keep any individual PE wait well under
the ~3.4us HAM window during the ramp.

Loop order is m-tile outer, k inner, n-group innermost: each
stationary x-slice is reused across the 2 moving n-groups and
LDWEIGHTS (135ns, no FWL in DoubleRow mode) hides under the 216ns
matmuls.
"""

import os

import numpy as np
import ml_dtypes

import concourse.bacc as bacc
import concourse.mybir as mybir
import concourse.tile as tile
from concourse.bass_utils import run_bass_kernel_spmd

M = K = N = 4096
X_SCALE, X_ZP = 0.03, -66
Y_SCALE, Y_ZP = 0.025, 160
S = X_SCALE * Y_SCALE
SQS = np.float32(np.sqrt(S))
CX = 65.5                 # x centering shift: x - X_ZP = (x + 0.5) + CX

NCORES = 8
MSPLIT, NSPLIT = 2, 4
MC = M // MSPLIT          # 2048 rows of C per core
NC = N // NSPLIT          # 1024 cols of C per core
P = 128                   # partitions
KC2 = K // (2 * P)        # 16 double-chunks (256 k-rows each)
MT = MC // P              # 16 m-tiles
MH = 2                    # x uploaded in MH m-halves (first sweep needs one)
MCH = MC // MH            # 1024 x-columns per half
NF = 512                  # matmul out free dim (one PSUM bank at fp32)
NG = NC // NF             # 2 n-groups
XQ = 1                    # k-double-chunks per x tile (256KB DMAs)
# y tile sizes in double-chunks (256KB each). Uniform small tiles won
# the clock lottery consistently: every variation that batched the
# upload into bigger or mixed-size DMAs measured the PE at ~2.0GHz
# (259ns/matmul) for the whole kernel instead of 2.4GHz (216ns).
YSIZES = [1] * KC2
YOFFS = np.cumsum([0] + YSIZES[:-1]).tolist()
assert sum(YSIZES) == KC2

FP8 = mybir.dt.float8e4
E4NP = ml_dtypes.float8_e4m3

WARM_MM = int(os.environ.get("K_WARM", "12"))
LAST_SPLIT = os.environ.get("K_LASTSPLIT", "1") == "1"

_CACHE = {}


def _build():
    nc = bacc.Bacc("TRN2", target_bir_lowering=False, debug=False)
    xt = nc.dram_tensor("xt", [MH, KC2 // XQ, P, 2 * XQ, MCH], FP8,
                        kind="ExternalInput")
    yts = [nc.dram_tensor(f"y{q}", [P, 2 * sz, NC], FP8, kind="ExternalInput")
           for q, sz in enumerate(YSIZES)]
    g = nc.dram_tensor("g", [P, NC], mybir.dt.float32, kind="ExternalInput")
    out = nc.dram_tensor("out", [MT, NG, P, NF], mybir.dt.float32,
                         kind="ExternalOutput")

    with tile.TileContext(nc) as tc:
        with (
            tc.tile_pool(name="warm_pool", bufs=1) as warm_pool,
            tc.tile_pool(name="xb_pool", bufs=MH * KC2 // XQ) as xb_pool,
            tc.tile_pool(name="yb_pool", bufs=len(YSIZES)) as yb_pool,
            tc.tile_pool(name="g_pool", bufs=1) as g_pool,
            tc.tile_pool(name="ot_pool", bufs=8) as ot_pool,
            tc.tile_pool(name="ps_pool", bufs=8, space="PSUM") as ps_pool,
        ):
            # PE warm-up: the upload ramp is DMA-heavy with little matmul
            # work; burn dummy matmuls so the HAM clock gate un-throttles
            # before the real matmuls issue.
            wt = warm_pool.tile([P, NF], mybir.dt.bfloat16, tag="wt")
            nc.vector.memset(wt[:], 0.0)
            wps = ps_pool.tile([64, NF], mybir.dt.float32, tag="ps", name="wps")
            for _ in range(WARM_MM):
                nc.tensor.matmul(wps[:], wt[:, :64], wt[:], start=True, stop=True)

            gt = g_pool.tile([P, NC], mybir.dt.float32, tag="gt")
            nc.gpsimd.dma_start(out=gt[:], in_=g[:, :])
            xb = [[xb_pool.tile([P, 2 * XQ, MCH], FP8, tag="xb",
                                name=f"xb{h}_{q}")
                   for q in range(KC2 // XQ)] for h in range(MH)]
            yb = [yb_pool.tile([P, 2 * sz, NC], FP8, tag="yb", name=f"yb{q}")
                  for q, sz in enumerate(YSIZES)]
            for q in range(len(YSIZES)):
                nc.scalar.dma_start(out=yb[q][:], in_=yts[q][:, :, :])
            for h in range(MH):
                for q in range(KC2 // XQ):
                    nc.sync.dma_start(out=xb[h][q][:], in_=xt[h, q, :, :, :])
            # k -> (y tile, chunk offset within tile)
            ymap = {}
            for q, (sz, off) in enumerate(zip(YSIZES, YOFFS)):
                for c in range(sz):
                    ymap[off + c] = (q, c)

            out_dma_engines = [nc.sync, nc.scalar]
            nout = [0]

            def drain(m, gi, psum):
                ot = ot_pool.tile([P, NF], mybir.dt.float32, tag="ot",
                                  name=f"ot{m}_{gi}")
                nc.vector.tensor_add(ot[:], psum[:],
                                     gt[:, gi * NF:(gi + 1) * NF])
                eng = out_dma_engines[nout[0] % 2]
                nout[0] += 1
                eng.dma_start(out=out[m, gi, :, :], in_=ot[:])

            def mm_sweep(m, groups, psums):
                h, mm = divmod(m, MT // MH)
                for k in range(KC2):
                    q, c = divmod(k, XQ)
                    xs = xb[h][q][:, 2 * c:2 * c + 2, mm * P:(mm + 1) * P]
                    yq, yc = ymap[k]
                    for j, gi in enumerate(groups):
                        nc.tensor.matmul(
                            psums[j][:], xs,
                            yb[yq][:, 2 * yc:2 * yc + 2,
                                   gi * NF:(gi + 1) * NF],
                            start=(k == 0), stop=(k == KC2 - 1),
                            perf_mode=mybir.MatmulPerfMode.DoubleRow)

            def ps_tiles(m, groups):
                return [ps_pool.tile([P, NF], mybir.dt.float32,
                                     tag="ps", name=f"ps{m}_{gi}")
                        for gi in groups]

            n_plain = MT - 1 if LAST_SPLIT else MT
            for m in range(n_plain):
                groups = list(range(NG))
                psums = ps_tiles(m, groups)
                mm_sweep(m, groups, psums)
                for gi in groups:
                    drain(m, gi, psums[gi])

            if LAST_SPLIT:
                m = MT - 1
                for gi in range(NG):
                    psums = ps_tiles(m, [gi])
                    mm_sweep(m, [gi], psums)
                    drain(m, gi, psums[0])
    nc.compile()
    return nc


def _get_nc():
    if "nc" not in _CACHE:
        _CACHE["nc"] = _build()
    return _CACHE["nc"]


def _chunk_block(a2d, ncols):
    """[K, ncols] -> [KC2, P, 2, ncols] with (c, p, i) -> k = 256c + 128i + p
    (the DoubleRow pairing)."""
    return np.ascontiguousarray(
        a2d.reshape(KC2, 2, P, ncols).transpose(0, 2, 1, 3))


def _shard(x, y):
    x = np.asarray(x, dtype=np.int32)
    y = np.asarray(y, dtype=np.int32)
    xq = ((x.astype(np.float32) + np.float32(0.5)) * SQS).astype(E4NP)
    yq = ((y.astype(np.float32) - np.float32(160.0)) * SQS).astype(E4NP)
    # exact column correction for the x centering shift
    gfull = (CX * S) * (y.astype(np.float64).sum(axis=0) - 160.0 * K)
    gfull = gfull.astype(np.float32)

    xts = []
    for mi in range(MSPLIT):
        blk = _chunk_block(
            np.ascontiguousarray(xq[mi * MC:(mi + 1) * MC, :].T), MC)
        # m-halves + quad-chunk tiles: [MH, KC2/XQ, P, 2*XQ, MCH]
        xts.append(np.ascontiguousarray(
            blk.reshape(KC2 // XQ, XQ, P, 2, MH, MCH)
            .transpose(4, 0, 2, 1, 3, 5)
            .reshape(MH, KC2 // XQ, P, 2 * XQ, MCH)))
    ys = []
    for ni in range(NSPLIT):
        blk = _chunk_block(
            np.ascontiguousarray(yq[:, ni * NC:(ni + 1) * NC]), NC)
        tiles = {}
        for q, (sz, off) in enumerate(zip(YSIZES, YOFFS)):
            # [sz, P, 2, NC] -> [P, 2*sz, NC]
            tiles[f"y{q}"] = np.ascontiguousarray(
                blk[off:off + sz].transpose(1, 0, 2, 3)
                .reshape(P, 2 * sz, NC))
        ys.append(tiles)
    gs = [np.ascontiguousarray(
              np.broadcast_to(gfull[ni * NC:(ni + 1) * NC], (P, NC)))
          for ni in range(NSPLIT)]
    in_maps = []
    for c in range(NCORES):
        mi, ni = divmod(c, NSPLIT)
        in_maps.append({"xt": xts[mi], "g": gs[ni], **ys[ni]})
    return in_maps


def _gather(results):
    out = np.empty((M, N), dtype=np.float32)
    for c in range(NCORES):
        mi, ni = divmod(c, NSPLIT)
        blk = results[c]["out"]  # [MT, NG, P, NF]
        out[mi * MC:(mi + 1) * MC, ni * NC:(ni + 1) * NC] = \
            blk.transpose(0, 2, 1, 3).reshape(MC, NC)
    return out


def run(x, y, **spmd_kwargs):
    """Run and return (full_output, BassKernelResults)."""
    nc = _get_nc()
    in_maps = _shard(x, y)
    res = run_bass_kernel_spmd(nc, in_maps, core_ids=list(range(NCORES)),
                               **spmd_kwargs)
    return _gather(res.results), res


def kernel(x, y):
    out, _ = run(x, y)
    return out
